# revision 13
# baseline (speedup 1.0000x reference)
"""Trainium2 Bass kernel for nn_ChannelPolyLayer.

out[b,o,x,y] = sum_c coeffs[b,o,c] * prod_v img[b,v,x,y] ** powers[c,v]
with degree<=3 trivariate monomials (20 coeffs), img channels (u,v,w).

Strategy (v5)
  - Data parallel over batch: 16 batches -> 8 cores x 2 batches; per core
    the 2 batches are folded onto the partition axis (rows 0..63 batch0,
    64..127 batch1), so per-partition scalar APs encode batch-dependent
    coefficients and one SPMD program serves all cores.
  - Factored evaluation  out = T(w) + u*A(u,v,w) + v*D(v,w)  where
        A: inhomogeneous quadratic (10 coeffs), D: quadratic in (v,w) (6),
        T: cubic in w alone incl. the global constant (4).
  - Completing-the-square: each (x^2, x) coefficient pair of A/D/T is
    evaluated as lam*Square(x + beta) + delta on the Scalar engine
    (f32 output; the cancellation lam*beta^2 is folded into delta inside a
    single fused tensor_scalar, so bf16 only ever rounds the small result).
  - Everything else runs as bf16 tensor_scalar leaves (DVE 4x mode) and
    bf16 tensor_tensor adds/mults (DVE 2x mode). No scalar_tensor_tensor
    (1x only on DVE) and no GpSimd (SBUF port contention poisons DVE).
  - ACT and DVE streams are balanced: ACT does the 18 squares + w^2 and
    most folds (Identity), DVE does cross/monomial leaves, trees, products.
"""

import numpy as np
import ml_dtypes

N_CORES = 8
BATCH, NVARS, H, W = 16, 3, 512, 512
NPIX = H * W            # 262144
P = 128
BPC = BATCH // N_CORES  # 2 batches per core
ROWS = P // BPC         # 64 partition rows per batch
COLS = NPIX // ROWS     # 4096 columns per plane
CW = 2048               # chunk width
NCHUNK = COLS // CW     # 2
NOUT = 3

# ctab column layout per output o (23 columns each, 69 total):
#   0..5   beta  for squares  [Au, Av, Aw, Dv, Dw, Ew]
#   6..11  lam   for folds    [same order]
#   12..17 delta for folds    [same order]
#   18 c_uv(A) 19 c_uw(A) 20 c_vw(A) 21 c_vw(D) 22 c000
TAB_PER_OUT = 23
TAB_COLS = NOUT * TAB_PER_OUT

# folds whose Identity runs on ACT instead of a DVE tensor_scalar
# (balance valve: ACT ~2.09us/op, DVE f32-in ts ~1.35us/op per 2048-chunk)
ACT_FOLDS = {(1, 0), (1, 1), (1, 2), (1, 3), (1, 4), (1, 5),
             (2, 0), (2, 1), (2, 2), (2, 3), (2, 4), (2, 5),
             (0, 4), (0, 5)}


def _fold_params(quad, lin, delta):
    """lam*Square(x+beta) + dlt  ==  quad*x^2 + lin*x + delta  (f64 host)."""
    aq, al = abs(quad), abs(lin)
    if aq < 1e-12 and al < 1e-12:
        return 0.0, 0.0, delta
    q = quad if aq >= al / 2000.0 else (al / 2000.0 if quad >= 0 else -al / 2000.0)
    beta = lin / (2.0 * q)
    lam = q
    return beta, lam, delta - lam * beta * beta


def _coeff_table(coeffs_core: np.ndarray, powers: np.ndarray) -> np.ndarray:
    """coeffs_core [BPC, NOUT, 20] -> ctab [P, TAB_COLS] f32."""
    pw = [tuple(int(round(x)) for x in row) for row in np.asarray(powers)]
    amap = {(0, 0, 0): 0, (1, 0, 0): 1, (0, 1, 0): 2, (0, 0, 1): 3,
            (2, 0, 0): 4, (1, 1, 0): 5, (1, 0, 1): 6, (0, 2, 0): 7,
            (0, 1, 1): 8, (0, 0, 2): 9}
    dmap = {(0, 0, 0): 0, (0, 1, 0): 1, (0, 0, 1): 2, (0, 2, 0): 3,
            (0, 1, 1): 4, (0, 0, 2): 5}
    out = np.empty((P, TAB_COLS), np.float32)
    for b in range(BPC):
        row = np.zeros(TAB_COLS, np.float64)
        for o in range(NOUT):
            A = np.zeros(10); D = np.zeros(6); T = np.zeros(4)
            for c, (pu, pv, pz) in enumerate(pw):
                val = float(coeffs_core[b, o, c])
                if pu > 0:
                    A[amap[(pu - 1, pv, pz)]] += val
                elif pv > 0:
                    D[dmap[(0, pv - 1, pz)]] += val
                else:
                    T[pz] += val
            base = o * TAB_PER_OUT
            pairs = [(A[4], A[1], A[0]),   # Au: u^2,u, const c100
                     (A[7], A[2], 0.0),    # Av
                     (A[9], A[3], 0.0),    # Aw
                     (D[3], D[1], D[0]),   # Dv: const c010
                     (D[5], D[2], 0.0),    # Dw
                     (T[3], T[2], T[1])]   # Ew: E = c001 + c002 w + c003 w^2
            for i, (q, l, d) in enumerate(pairs):
                beta, lam, dlt = _fold_params(q, l, d)
                row[base + i] = beta
                row[base + 6 + i] = lam
                row[base + 12 + i] = dlt
            row[base + 18] = A[5]  # uv
            row[base + 19] = A[6]  # uw
            row[base + 20] = A[8]  # vw (A)
            row[base + 21] = D[4]  # vw (D)
            row[base + 22] = T[0]  # c000
        out[b * ROWS:(b + 1) * ROWS, :] = row.astype(np.float32)
    return out


_NC_CACHE = {}

# Dev knobs (unused by the grading harness): extra kwargs forwarded to
# run_bass_kernel_spmd, and the last BassKernelResults for inspection.
RUN_KWARGS: dict = {}
LAST_RESULTS = None


def _build_nc():
    if "nc" in _NC_CACHE:
        return _NC_CACHE["nc"]
    import concourse.mybir as mybir
    from concourse import bacc
    from concourse.tile import TileContext

    F32 = mybir.dt.float32
    BF16 = mybir.dt.bfloat16
    MUL = mybir.AluOpType.mult
    ADD = mybir.AluOpType.add
    IDENT = mybir.ActivationFunctionType.Identity
    SQ = mybir.ActivationFunctionType.Square

    nc = bacc.Bacc("TRN2", target_bir_lowering=False)
    img = nc.dram_tensor("img", [NVARS, P, COLS], BF16, kind="ExternalInput")
    ctab = nc.dram_tensor("ctab", [P, TAB_COLS], F32, kind="ExternalInput")
    out = nc.dram_tensor("out", [NOUT, P, COLS], BF16, kind="ExternalOutput")

    with TileContext(nc) as tc:
        with (
            tc.tile_pool(name="tabp", bufs=1) as tabp,
            tc.tile_pool(name="inp", bufs=2) as inp,
            tc.tile_pool(name="crs", bufs=2) as crs,
            tc.tile_pool(name="sqp", bufs=4) as sqp,
            tc.tile_pool(name="leafp", bufs=6) as leafp,
            tc.tile_pool(name="accp", bufs=6) as accp,
            tc.tile_pool(name="outp", bufs=2) as outp,
        ):
            tab = tabp.tile([P, TAB_COLS], F32)
            nc.sync.dma_start(out=tab, in_=ctab[:, :])

            def col(o, k):
                j = o * TAB_PER_OUT + k
                return tab[:, j:j + 1]

            for chk in range(NCHUNK):
                c0, c1 = chk * CW, (chk + 1) * CW
                ub = inp.tile([P, CW], BF16, tag="ub")
                vb = inp.tile([P, CW], BF16, tag="vb")
                wb = inp.tile([P, CW], BF16, tag="wb")
                nc.sync.dma_start(out=ub, in_=img[0, :, c0:c1])
                nc.sync.dma_start(out=vb, in_=img[1, :, c0:c1])
                nc.sync.dma_start(out=wb, in_=img[2, :, c0:c1])

                uv = crs.tile([P, CW], BF16, tag="uv")
                uw = crs.tile([P, CW], BF16, tag="uw")
                vw = crs.tile([P, CW], BF16, tag="vw")
                nc.vector.tensor_tensor(out=uv, in0=ub, in1=vb, op=MUL)
                nc.vector.tensor_tensor(out=uw, in0=ub, in1=wb, op=MUL)
                nc.vector.tensor_tensor(out=vw, in0=vb, in1=wb, op=MUL)

                sq_in = [ub, vb, wb, vb, wb, wb]  # Au Av Aw Dv Dw Tw
                for o in range(NOUT):
                    # squares on ACT (f32 out), folds fused lam*sq+delta
                    folds = []
                    for i in range(6):
                        sq = sqp.tile([P, CW], F32, tag="sq",
                                      name=f"sq{o}_{i}_{chk}")
                        nc.scalar.activation(out=sq, in_=sq_in[i], func=SQ,
                                             bias=col(o, i))
                        fl = leafp.tile([P, CW], BF16, tag="leaf",
                                        name=f"fold{o}_{i}_{chk}")
                        if (o, i) in ACT_FOLDS:
                            nc.scalar.activation(out=fl, in_=sq, func=IDENT,
                                                 scale=col(o, 6 + i),
                                                 bias=col(o, 12 + i))
                        else:
                            nc.vector.tensor_scalar(
                                out=fl, in0=sq, scalar1=col(o, 6 + i),
                                scalar2=col(o, 12 + i), op0=MUL, op1=ADD)
                        folds.append(fl)

                    def leaf(src, k, nm):
                        t = leafp.tile([P, CW], BF16, tag="leaf", name=nm)
                        nc.vector.tensor_scalar(out=t, in0=src,
                                                scalar1=col(o, k),
                                                scalar2=0.0, op0=MUL, op1=ADD)
                        return t

                    def tt(a, b, nm, op=ADD, pool=accp, tag="acc"):
                        t = pool.tile([P, CW], BF16, tag=tag, name=nm)
                        nc.vector.tensor_tensor(out=t, in0=a, in1=b, op=op)
                        return t

                    # running chains; leaves emitted adjacent to their use
                    at = tt(folds[0], folds[1], f"a1_{o}_{chk}")
                    at = tt(at, folds[2], f"a2_{o}_{chk}")
                    luv = leaf(uv, 18, f"luv{o}_{chk}")
                    at = tt(at, luv, f"a3_{o}_{chk}")
                    luw = leaf(uw, 19, f"luw{o}_{chk}")
                    at = tt(at, luw, f"a4_{o}_{chk}")
                    lvwA = leaf(vw, 20, f"lvwA{o}_{chk}")
                    at = tt(at, lvwA, f"a5_{o}_{chk}")
                    dt = tt(folds[3], folds[4], f"d1_{o}_{chk}")
                    lvwD = leaf(vw, 21, f"lvwD{o}_{chk}")
                    dt = tt(dt, lvwD, f"d2_{o}_{chk}")
                    p1 = tt(at, ub, f"p1_{o}_{chk}", op=MUL)
                    p2 = tt(dt, vb, f"p2_{o}_{chk}", op=MUL)
                    p3 = tt(folds[5], wb, f"p3_{o}_{chk}", op=MUL)
                    s = tt(p1, p2, f"s_{o}_{chk}")
                    s2 = tt(s, p3, f"s2_{o}_{chk}")
                    ot = outp.tile([P, CW], BF16, tag=f"ot{o}",
                                   name=f"ot_{o}_{chk}")
                    nc.vector.tensor_scalar(out=ot, in0=s2, scalar1=1.0,
                                            scalar2=col(o, 22), op0=MUL,
                                            op1=ADD)
                    nc.sync.dma_start(out=out[o, :, c0:c1], in_=ot)
    nc.finalize()
    _NC_CACHE["nc"] = nc
    return nc


def _shard_core(img_bf: np.ndarray, c: int) -> np.ndarray:
    """img_bf [BATCH,3,H,W] bf16 -> per-core [NVARS, P, COLS]."""
    blk = np.empty((NVARS, P, COLS), ml_dtypes.bfloat16)
    for b in range(BPC):
        plane = img_bf[c * BPC + b].reshape(NVARS, ROWS, COLS)
        blk[:, b * ROWS:(b + 1) * ROWS, :] = plane
    return blk


def kernel(img: np.ndarray, coeffs: np.ndarray, powers: np.ndarray) -> np.ndarray:
    from concourse.bass_utils import run_bass_kernel_spmd

    img_bf = np.asarray(img, np.float32).astype(ml_dtypes.bfloat16)
    coeffs = np.asarray(coeffs, np.float32)
    powers = np.asarray(powers, np.float32)

    nc = _build_nc()
    in_maps = []
    for c in range(N_CORES):
        in_maps.append({
            "img": _shard_core(img_bf, c),
            "ctab": _coeff_table(coeffs[c * BPC:(c + 1) * BPC], powers),
        })

    res = run_bass_kernel_spmd(nc, in_maps, core_ids=list(range(N_CORES)),
                               **RUN_KWARGS)
    global LAST_RESULTS
    LAST_RESULTS = res
    out = np.empty((BATCH, NOUT, H, W), np.float32)
    for c in range(N_CORES):
        blk = np.asarray(res.results[c]["out"], dtype=np.float32)
        for b in range(BPC):
            out[c * BPC + b] = blk[:, b * ROWS:(b + 1) * ROWS, :].reshape(
                NOUT, H, W)
    return out
